# revision 1
# baseline (speedup 1.0000x reference)
"""L-BFGS two-loop recursion (apply_Hv) on 8 Trainium2 NeuronCores.

Vector-free reformulation: instead of 60 sequential dot-product/axpy steps
(each of which would need its own scalar AllReduce at a ~10us floor), the
two-loop recursion is algebraically equivalent to

  1. Gram pass   : G = [S; Y; v] @ [Y; v]^T            (one streaming pass)
  2. tiny scalar : alpha/beta recursions on the 61x31 Gram entries
  3. combine pass: res = (a/theta) v - sum_j (alpha_j/theta) y_j
                       + sum_j (alpha_j - beta_j) s_j  (one streaming pass)

The n dimension is sharded across the 8 cores; one 7.6 KB AllReduce merges
the per-core partial Gram matrices.  Both passes are HBM-bandwidth bound.

Per core, phase A streams natural-layout [122, F] tiles (rows = S(30), Y(30),
v packed twice along partitions for full DMA width), transposes 128-column
blocks on the TensorEngine (fp32 needs the identity-matmul path), and
accumulates the Gram matrix in PSUM.  Phase D is a per-row fused
scalar_tensor_tensor accumulation on the VectorEngine.
"""

import numpy as np

import concourse.bass as bass
import concourse.mybir as mybir
from concourse import bacc
from concourse.bass_utils import run_bass_kernel_spmd
from concourse.masks import make_identity
from concourse.tile import TileContext

F32 = mybir.dt.float32
M = 30  # L-BFGS history length
X = 2 * M + 1  # rows of [S; Y; v]
NCORES = 8
N_FULL = 4_194_304
N_CORE = N_FULL // NCORES

# phase A: one chunk covers CH_A consecutive n per core, laid out as
# [122, F_A] (two n-halves stacked along partitions)
F_A = 4096
CH_A = 2 * F_A


def build_kernel(n_core: int = N_CORE, n_cores: int = NCORES):
    assert n_core % CH_A == 0
    n_chunks_a = n_core // CH_A
    # phase D: [128, f_d] tiles, n-chunk = 128 * f_d
    f_d = min(2048, n_core // 128)
    ch_d = 128 * f_d
    assert n_core % ch_d == 0
    n_chunks_d = n_core // ch_d

    nc = bacc.Bacc(None, target_bir_lowering=False, debug=False)

    s_d = nc.declare_dram_parameter("s_s", [M, n_core], F32, isOutput=False)
    y_d = nc.declare_dram_parameter("y_s", [M, n_core], F32, isOutput=False)
    v_d = nc.declare_dram_parameter("v_s", [n_core], F32, isOutput=False)
    ys_d = nc.declare_dram_parameter("ys", [M], F32, isOutput=False)
    th_d = nc.declare_dram_parameter("theta", [1], F32, isOutput=False)
    a_d = nc.declare_dram_parameter("a", [1], F32, isOutput=False)
    out_d = nc.declare_dram_parameter("out", [n_core], F32, isOutput=True)

    g_loc = nc.dram_tensor("g_loc", [X, M + 1], F32)
    g_red = nc.dram_tensor("g_red", [X, M + 1], F32, addr_space="Shared")

    add = mybir.AluOpType.add
    mult = mybir.AluOpType.mult
    subtract = mybir.AluOpType.subtract

    with TileContext(nc) as tc:
        with (
            tc.tile_pool(name="consts", bufs=1) as consts,
            tc.tile_pool(name="x2", bufs=3) as x2_pool,
            tc.tile_pool(name="xt", bufs=4) as xt_pool,
            tc.tile_pool(name="pstr", bufs=3, space="PSUM") as pstr_pool,
            tc.tile_pool(name="psg", bufs=2, space="PSUM") as psg_pool,
            tc.tile_pool(name="small", bufs=1) as small,
            tc.tile_pool(name="dacc", bufs=2) as dacc_pool,
            tc.tile_pool(name="drow", bufs=10) as drow_pool,
            tc.tile_pool(name="dv", bufs=2) as dv_pool,
        ):
            identity = consts.tile([122, 122], F32)
            make_identity(nc, identity)

            # ---------------- phase A: Gram matrix ----------------
            g_acc = small.tile([X, M + 1], F32)
            nc.vector.memset(g_acc, 0.0)

            for c in range(n_chunks_a):
                n0 = c * CH_A
                x2 = x2_pool.tile([2 * X, F_A], F32, tag="x2")
                # partition p = h*61 + j holds row j of [S;Y;v], n-half h.
                # plain partition-range slices only (nested partition dims in
                # one SBUF DMA dst are not supported).
                for h in range(2):
                    nh = n0 + h * F_A
                    nc.sync.dma_start(
                        out=x2[h * X : h * X + M, :],
                        in_=s_d[:, nh : nh + F_A],
                    )
                    nc.sync.dma_start(
                        out=x2[h * X + M : h * X + 2 * M, :],
                        in_=y_d[:, nh : nh + F_A],
                    )
                    nc.sync.dma_start(
                        out=x2[h * X + 2 * M : h * X + X, :],
                        in_=v_d[nh : nh + F_A].rearrange("(o f) -> o f", o=1),
                    )

                gps = psg_pool.tile([X, M + 1], F32, tag="gps")
                n_blk = F_A // 128  # 128-column transpose blocks
                mm = 0
                for q in range(n_blk // 4):
                    ps = pstr_pool.tile([128, 4, 2 * X], F32, tag="pstr")
                    for t in range(4):
                        b = q * 4 + t
                        nc.tensor.transpose(
                            ps[:, t, :], x2[:, b * 128 : (b + 1) * 128], identity
                        )
                    xt = xt_pool.tile([128, 4, 2 * X], F32, tag="xt")
                    nc.any.tensor_copy(xt, ps)
                    for t in range(4):
                        for h in range(2):
                            nc.tensor.matmul(
                                gps,
                                xt[:, t, h * X : (h + 1) * X],
                                xt[:, t, h * X + M : (h + 1) * X],
                                start=(mm == 0),
                                stop=(mm == 8 * (n_blk // 4) - 1),
                            )
                            mm += 1
                nc.vector.tensor_tensor(out=g_acc, in0=g_acc, in1=gps, op=add)

            # ---------------- phase B: AllReduce ----------------
            nc.sync.dma_start(out=g_loc[:, :], in_=g_acc)
            nc.gpsimd.collective_compute(
                "AllReduce",
                add,
                ins=[g_loc[:, :]],
                outs=[g_red[:, :]],
                replica_groups=[list(range(n_cores))],
            )

            # ---------------- phase C: scalar recursions ----------------
            # everything on partition 0; G flattened to [1, X*(M+1)]
            W = M + 1
            gf = small.tile([1, X * W], F32)
            nc.sync.dma_start(
                out=gf, in_=g_red[:, :].rearrange("(o a) b -> o (a b)", o=1)
            )
            ys_t = small.tile([1, M], F32)
            nc.sync.dma_start(out=ys_t, in_=ys_d[:].rearrange("(o a) -> o a", o=1))
            a_t = small.tile([1, 1], F32)
            nc.sync.dma_start(out=a_t, in_=a_d[:].rearrange("(o a) -> o a", o=1))
            th_t = small.tile([1, 1], F32)
            nc.sync.dma_start(out=th_t, in_=th_d[:].rearrange("(o a) -> o a", o=1))

            inv_ys = small.tile([1, M], F32)
            nc.vector.reciprocal(inv_ys, ys_t)
            inv_th = small.tile([1, 1], F32)
            nc.vector.reciprocal(inv_th, th_t)

            coeff = small.tile([1, 2 * M + 2], F32)  # [c_s(30) | c_y(30) | c_v | pad]
            alpha = small.tile([1, M], F32)
            u_row = small.tile([1, M], F32)
            nc.vector.memset(u_row, 0.0)
            tmp_r = small.tile([1, M], F32)
            dotn = small.tile([1, 1], F32)
            tsc = small.tile([1, 1], F32)

            def sy(j):  # s_j . y_k row
                return gf[:, j * W : j * W + M]

            def yy(j):
                return gf[:, (M + j) * W : (M + j) * W + M]

            sv = lambda j: gf[:, j * W + M : j * W + M + 1]
            yv = lambda j: gf[:, (M + j) * W + M : (M + j) * W + M + 1]

            # loop 1: alpha_j = (a*Sv_j - sum_{k>j} SY[j,k] alpha_k) / ys_j
            for j in range(M - 1, -1, -1):
                if j < M - 1:
                    nk = M - 1 - j
                    nc.vector.tensor_tensor(
                        out=tmp_r[:, :nk],
                        in0=gf[:, j * W + j + 1 : j * W + M],
                        in1=alpha[:, j + 1 : M],
                        op=mult,
                    )
                    nc.vector.tensor_reduce(
                        out=dotn, in_=tmp_r[:, :nk],
                        axis=mybir.AxisListType.X, op=add, negate=True,
                    )
                    nc.vector.scalar_tensor_tensor(
                        out=tsc, in0=sv(j), scalar=a_t, in1=dotn, op0=mult, op1=add
                    )
                else:
                    nc.vector.tensor_scalar(
                        out=tsc, in0=sv(j), scalar1=a_t, scalar2=None, op0=mult
                    )
                nc.vector.tensor_tensor(
                    out=alpha[:, j : j + 1], in0=tsc, in1=inv_ys[:, j : j + 1], op=mult
                )

            # loop 2: beta_j = (w_j/theta + u_j) / ys_j ;  d_j = alpha_j - beta_j
            # w_j = a*Yv_j - sum_k YY[j,k] alpha_k ;  u accumulates d_k * SY[k, :]
            for j in range(M):
                nc.vector.tensor_tensor(out=tmp_r, in0=yy(j), in1=alpha, op=mult)
                nc.vector.tensor_reduce(
                    out=dotn, in_=tmp_r, axis=mybir.AxisListType.X, op=add, negate=True
                )
                nc.vector.scalar_tensor_tensor(
                    out=tsc, in0=yv(j), scalar=a_t, in1=dotn, op0=mult, op1=add
                )
                nc.vector.scalar_tensor_tensor(
                    out=tsc, in0=tsc, scalar=inv_th, in1=u_row[:, j : j + 1],
                    op0=mult, op1=add,
                )
                nc.vector.tensor_tensor(
                    out=tsc, in0=tsc, in1=inv_ys[:, j : j + 1], op=mult
                )  # beta_j
                nc.vector.tensor_tensor(
                    out=coeff[:, j : j + 1], in0=alpha[:, j : j + 1], in1=tsc,
                    op=subtract,
                )  # d_j = c_s[j]
                if j < M - 1:
                    nc.vector.scalar_tensor_tensor(
                        out=u_row, in0=sy(j), scalar=coeff[:, j : j + 1], in1=u_row,
                        op0=mult, op1=add,
                    )

            # c_y = -alpha/theta ; c_v = a/theta
            nc.vector.tensor_scalar(
                out=coeff[:, M : 2 * M], in0=alpha, scalar1=inv_th, scalar2=-1.0,
                op0=mult, op1=mult,
            )
            nc.vector.tensor_scalar(
                out=coeff[:, 2 * M : 2 * M + 1], in0=a_t, scalar1=inv_th,
                scalar2=None, op0=mult,
            )

            # broadcast coeff row to all 128 partitions: ones[128]^T outer coeff
            ones_t = consts.tile([1, 128], F32)
            nc.vector.memset(ones_t, 1.0)
            cb_ps = psg_pool.tile([128, X], F32, tag="cbps")
            nc.tensor.matmul(cb_ps, ones_t, coeff[:, :X], start=True, stop=True)
            c_full = small.tile([128, X], F32)
            nc.any.tensor_copy(c_full, cb_ps)

            # ---------------- phase D: res = sum_j coeff_j * row_j ----------------
            for c in range(n_chunks_d):
                n0 = c * ch_d
                acc = dacc_pool.tile([128, f_d], F32, tag="dacc")
                vch = dv_pool.tile([128, f_d], F32, tag="dv")
                nc.sync.dma_start(
                    out=vch,
                    in_=v_d[n0 : n0 + ch_d].rearrange("(p f) -> p f", p=128),
                )
                nc.vector.tensor_scalar(
                    out=acc, in0=vch, scalar1=c_full[:, X - 1 : X], scalar2=None,
                    op0=mult,
                )
                for src, coff in ((s_d, 0), (y_d, M)):
                    for j in range(M):
                        row = drow_pool.tile([128, f_d], F32, tag="drow")
                        nc.sync.dma_start(
                            out=row,
                            in_=src[j, n0 : n0 + ch_d].rearrange(
                                "(p f) -> p f", p=128
                            ),
                        )
                        nc.vector.scalar_tensor_tensor(
                            out=acc, in0=row, scalar=c_full[:, coff + j : coff + j + 1],
                            in1=acc, op0=mult, op1=add,
                        )
                nc.sync.dma_start(
                    out=out_d[n0 : n0 + ch_d].rearrange("(p f) -> p f", p=128),
                    in_=acc,
                )

    nc.compile()
    return nc


_BUILD_CACHE = {}


def _get_nc(n_core: int, n_cores: int):
    key = (n_core, n_cores)
    if key not in _BUILD_CACHE:
        _BUILD_CACHE[key] = build_kernel(n_core, n_cores)
    return _BUILD_CACHE[key]


def run(v, s, y, ys, theta, a, trace=False):
    n = v.shape[0]
    n_core = n // NCORES
    nc = _get_nc(n_core, NCORES)
    in_maps = []
    for c in range(NCORES):
        sl = slice(c * n_core, (c + 1) * n_core)
        in_maps.append(
            {
                "s_s": np.ascontiguousarray(s[:, sl]),
                "y_s": np.ascontiguousarray(y[:, sl]),
                "v_s": np.ascontiguousarray(v[sl]),
                "ys": np.ascontiguousarray(ys),
                "theta": np.asarray(theta, dtype=np.float32).reshape(1),
                "a": np.asarray(a, dtype=np.float32).reshape(1),
            }
        )
    res = run_bass_kernel_spmd(nc, in_maps, list(range(NCORES)), trace=trace)
    out = np.concatenate([res.results[c]["out"] for c in range(NCORES)])
    return out, res


def kernel(v, s, y, ys, theta, a):
    out, _ = run(
        np.asarray(v, np.float32),
        np.asarray(s, np.float32),
        np.asarray(y, np.float32),
        np.asarray(ys, np.float32),
        theta,
        a,
    )
    return out

